# revision 21
# baseline (speedup 1.0000x reference)
"""ClusterAttention2 Trainium2 kernel.

Mathematical simplification: the reference computes
    logits       : [n_clusters, 1]
    att_clusters = softmax(logits, axis=1)   # axis of size 1 -> exactly ones
    att_vertices = adj.T @ att_clusters      # == per-vertex column sum of adj
    att_vertices = att_vertices / max(att_vertices, axis=1)  # [N,1] -> x/x
so for any finite logits the output is exactly
    att_clusters = ones([n_clusters, 1])
    att_vertices = colsum / colsum           # 1.0, or NaN where colsum == 0
The only data-dependent work is the column sum of adj (400 MB -> memory
bound).  Each of the 8 cores reads a [1000, 12500] vertex-shard of adj and
reduces the cluster dimension on the tensor engine (ones[128,1].T @
adj_tile[128,500], accumulated over the 8 cluster chunks in PSUM).  The
final x/x division (IEEE 0/0 -> NaN) runs on the host so NaN positions
match the reference bit-for-bit.

Written in raw Bass (explicit semaphores): the TPB ISA allows a single
semaphore wait per instruction, so every wait is a standalone wait_ge on
the consuming engine, never attached to a data instruction.

Pipeline (per core), vertex blocks vb = 0..4 of width 2500:
  SP   : input DMAs adj[k*128:.., vb*2500:..] -> sbuf slot (vb%2, k),
         gated on s_pe so a slot is only overwritten after consumption.
  ACT  : the 5 output DMAs obuf[vb%2] -> cs, gated on s_cp.
  PE   : 25 accumulation groups (vb, i): 8 matmuls each, start/stop in a
         rotating PSUM bank; group (vb, 0) waits for ALL 8 tiles of the
         block (s_in >= 16*8*(vb+1)) — completion-count safe because no
         DMA of block vb+1 can start before PE passes this point.
  DVE  : memset ones; copy each finished PSUM group into obuf[vb%2].

`repeat` > 1 replays the whole pipeline (same data, same output) for
slope-based hardware timing through the high-overhead dispatch path.
"""

import numpy as np

import concourse.bass as bass
import concourse.mybir as mybir
from concourse.bass_utils import run_bass_kernel_spmd

N_CLUSTERS = 1000
N_VERTICES = 100000
N_CORES = 8
V_SHARD = N_VERTICES // N_CORES  # 12500 vertices per core
P = 128                          # cluster chunk (partition dim)
N_K = (N_CLUSTERS + P - 1) // P  # 8 chunks: 7x128 + 104
F_DMA = 2500                     # vertices per DMA tile (128x2500 f32 = 1.25 MB)
F_MM = 500                       # vertices per matmul (PSUM bank = 512 f32)
N_VB = V_SHARD // F_DMA          # 5 vertex blocks
N_I = F_DMA // F_MM              # 5 accumulation groups per block
N_PS = 4                         # rotating PSUM banks


def _build_nc(repeat: int = 1) -> bass.Bass:
    nc = bass.Bass()
    adj_s = nc.dram_tensor(
        "adj_s", [N_CLUSTERS, V_SHARD], mybir.dt.float32, kind="ExternalInput"
    )
    cs = nc.dram_tensor("cs", [V_SHARD], mybir.dt.float32, kind="ExternalOutput")
    NV = N_VB * repeat  # global vertex-block count

    with (
        nc.sbuf_tensor([P, 2 * N_K * F_DMA], mybir.dt.float32) as tbuf,
        nc.sbuf_tensor([P, F_DMA], mybir.dt.float32) as acc,
        nc.sbuf_tensor([P, 1], mybir.dt.float32) as ones,
        nc.sbuf_tensor([1, 2 * F_DMA], mybir.dt.float32) as obuf,
        # 512-f32 stride so each rotating accumulator is bank-aligned
        nc.psum_tensor([1, N_PS, 512], mybir.dt.float32) as pst,
        nc.semaphore("s_init") as s_init,
        # input-completion sems split by block parity: a consumer threshold
        # then only ever counts DMAs of blocks <= v of that parity (blocks
        # v+2 of the same parity are gated on adds of block v), so a
        # straggler from an adjacent block can never satisfy the wait.
        nc.semaphore("s_in0") as s_in0,
        nc.semaphore("s_in1") as s_in1,
        nc.semaphore("s_add") as s_add,
        nc.semaphore("s_pe") as s_pe,
        nc.semaphore("s_cp") as s_cp,
        nc.semaphore("s_out") as s_out,
        nc.Block() as block,
    ):
        s_in = [s_in0, s_in1]

        def tslot(v, k):
            return tbuf[:, ((v % 2) * N_K + k) * F_DMA : ((v % 2) * N_K + k + 1) * F_DMA]

        def kp_of(k):
            return min(P, N_CLUSTERS - k * P)

        # clusters 0:896 of a block, as one DMA: DRAM rows (k*128+p) map to
        # SBUF (p, k*2500+w), which is contiguous free-dim on the SBUF side
        adj_kp = adj_s[: (N_K - 1) * P, :].rearrange("(k p) w -> p k w", p=P)

        @block.sync
        def _(sync):
            for v in range(NV):
                vb = v % N_VB
                if v >= 2:
                    # slots (v%2, *) free once DVE summed all of block v-2
                    sync.wait_ge(s_add, v - 1)
                # k = 0..6 fused (896 rows), then the 104-row tail chunk
                sync.dma_start(
                    out=tbuf[:, (v % 2) * N_K * F_DMA : ((v % 2) * N_K + N_K - 1) * F_DMA],
                    in_=adj_kp[:, 0 : N_K - 1, vb * F_DMA : (vb + 1) * F_DMA],
                ).then_inc(s_in[v % 2], 16)
                kp = kp_of(N_K - 1)
                sync.dma_start(
                    out=tslot(v, N_K - 1)[:kp, :],
                    in_=adj_s[(N_K - 1) * P : (N_K - 1) * P + kp, vb * F_DMA : (vb + 1) * F_DMA],
                ).then_inc(s_in[v % 2], 16)

        @block.gpsimd
        def _(gpsimd):
            for v in range(NV):
                vb = v % N_VB
                gpsimd.wait_ge(s_cp, N_I * (v + 1))
                gpsimd.dma_start(
                    out=cs[vb * F_DMA : (vb + 1) * F_DMA],
                    in_=obuf[:1, (v % 2) * F_DMA : (v % 2 + 1) * F_DMA],
                ).then_inc(s_out, 16)
            gpsimd.wait_ge(s_out, 16 * NV)

        @block.tensor
        def _(tensor):
            tensor.wait_ge(s_init, 1)  # ones ready
            for v in range(NV):
                # acc for block v complete
                tensor.wait_ge(s_add, v + 1)
                for i in range(N_I):
                    g = v * N_I + i
                    if g >= N_PS:
                        # PSUM bank g%N_PS free once copy of group g-N_PS done
                        tensor.wait_ge(s_cp, g - N_PS + 1)
                    nc.tensor.matmul(
                        pst[:1, g % N_PS, :F_MM],
                        ones[:, :1],
                        acc[:, i * F_MM : (i + 1) * F_MM],
                        start=True,
                        stop=True,
                    ).then_inc(s_pe, 1)

        @block.vector
        def _(vector):
            vector.memset(ones[:, :], 1.0).then_inc(s_init, 1)
            for v in range(NV):
                # acc free once PE consumed block v-1's acc
                if v >= 1:
                    vector.wait_ge(s_pe, N_I * v)
                # both DMAs of this block arrived (fused chunk could finish
                # after the smaller tail chunk, so count them together)
                vector.wait_ge(s_in[v % 2], 32 * (v // 2 + 1))
                # acc = sum of the 8 cluster chunks (tail chunk is 104 rows)
                add = nc.vector.tensor_add(acc[:, :], tslot(v, 0)[:, :], tslot(v, 1)[:, :])
                for k in range(2, N_K):
                    kp = kp_of(k)
                    add = nc.vector.tensor_add(
                        acc[:kp, :], acc[:kp, :], tslot(v, k)[:kp, :]
                    )
                add.then_inc(s_add, 1)
                for i in range(N_I):
                    g = v * N_I + i
                    if i == 0 and v >= 2:
                        # obuf half (v%2) free once output DMA of v-2 done
                        vector.wait_ge(s_out, 16 * (v - 1))
                    vector.wait_ge(s_pe, g + 1)
                    nc.vector.tensor_copy(
                        obuf[:1, (v % 2) * F_DMA + i * F_MM : (v % 2) * F_DMA + (i + 1) * F_MM],
                        pst[:1, g % N_PS, :F_MM],
                    ).then_inc(s_cp, 1)

    return nc


def kernel(x, adj, att, key_w):
    adj = np.ascontiguousarray(np.asarray(adj), dtype=np.float32)
    assert adj.shape == (N_CLUSTERS, N_VERTICES)

    nc = _build_nc()
    in_maps = [
        {"adj_s": np.ascontiguousarray(adj[:, i * V_SHARD : (i + 1) * V_SHARD])}
        for i in range(N_CORES)
    ]
    res = run_bass_kernel_spmd(nc, in_maps, core_ids=list(range(N_CORES)))
    colsum = np.concatenate([r["cs"] for r in res.results]).astype(np.float32)

    with np.errstate(divide="ignore", invalid="ignore"):
        att_vertices = (colsum / colsum).reshape(N_VERTICES, 1).astype(np.float32)
    att_clusters = np.ones((N_CLUSTERS, 1), dtype=np.float32)
    return att_vertices, att_clusters


# revision 23
# speedup vs baseline: 717.9019x; 717.9019x over previous
"""ClusterAttention2 Trainium2 kernel.

Mathematical simplification: the reference computes
    logits       : [n_clusters, 1]
    att_clusters = softmax(logits, axis=1)   # axis of size 1 -> exactly ones
    att_vertices = adj.T @ att_clusters      # == per-vertex column sum of adj
    att_vertices = att_vertices / max(att_vertices, axis=1)  # [N,1] -> x/x
so for any finite logits the output is exactly
    att_clusters = ones([n_clusters, 1])
    att_vertices = colsum / colsum           # 1.0, or NaN where colsum == 0
The only data-dependent work is the column sum of adj (400 MB -> memory
bound).  Each of the 8 cores reads its [1000, 12500] vertex-shard of adj
(50 MB, the per-core HBM roofline at ~358 GB/s is ~140 us) and reduces the
cluster dimension in two steps: DVE sums the 8 row-chunks of 128 clusters
lane-wise into acc[128, 2500], then one tensor-engine matmul per 500
columns (ones[128,1].T @ acc) folds the remaining 128 partitions.  The
final x/x division (IEEE 0/0 -> NaN) runs on the host so NaN positions
match the reference bit-for-bit.

Written in raw Bass (explicit semaphores): the TPB ISA allows a single
semaphore wait per instruction (this walrus build hard-errors on more), so
every wait is a standalone wait_ge on the consuming engine, never attached
to a data instruction.

Pipeline (per core), vertex blocks v of width 2500 (5 per pass):
  SP   : 2 input DMAs per block into sbuf half (v%2) — one fused
         [128, 7x2500] transfer for cluster rows 0:896 (DRAM rows k*128+p
         map to SBUF (p, k*2500+w), contiguous on the SBUF side) and one
         for the 104-row tail; gated on s_add so a half is only
         overwritten after DVE consumed it.
  DVE  : acc = sum of the block's 8 chunks (7 tensor_adds; fused and tail
         chunks have separate parity-split completion sems so a count can
         never be satisfied by the wrong DMA finishing first), then copy
         each finished PSUM group into obuf[v%2].
  PE   : 5 single matmuls per block (K=128) into a rotating PSUM bank.
  POOL : output DMA obuf[v%2] -> cs per block, gated on s_cp.

`repeat` > 1 replays the whole pipeline (same data, same output) for
slope-based hardware timing through the high-overhead dispatch path.
"""

import numpy as np

import concourse.bass as bass
import concourse.mybir as mybir
from concourse.bass_utils import run_bass_kernel_spmd

N_CLUSTERS = 1000
N_VERTICES = 100000
N_CORES = 8
V_SHARD = N_VERTICES // N_CORES  # 12500 vertices per core
P = 128                          # cluster chunk (partition dim)
N_K = (N_CLUSTERS + P - 1) // P  # 8 chunks: 7x128 + 104
F_DMA = 2500                     # vertices per DMA tile (128x2500 f32 = 1.25 MB)
F_MM = 500                       # vertices per matmul (PSUM bank = 512 f32)
N_VB = V_SHARD // F_DMA          # 5 vertex blocks
N_I = F_DMA // F_MM              # 5 accumulation groups per block
N_PS = 4                         # rotating PSUM banks


def _build_nc(repeat: int = 1) -> bass.Bass:
    nc = bass.Bass()
    adj_s = nc.dram_tensor(
        "adj_s", [N_CLUSTERS, V_SHARD], mybir.dt.float32, kind="ExternalInput"
    )
    cs = nc.dram_tensor("cs", [V_SHARD], mybir.dt.float32, kind="ExternalOutput")
    NV = N_VB * repeat  # global vertex-block count

    with (
        nc.sbuf_tensor([P, 2 * N_K * F_DMA], mybir.dt.float32) as tbuf,
        nc.sbuf_tensor([P, F_DMA], mybir.dt.float32) as acc,
        nc.sbuf_tensor([P, 1], mybir.dt.float32) as ones,
        nc.sbuf_tensor([1, 2 * F_DMA], mybir.dt.float32) as obuf,
        # 512-f32 stride so each rotating accumulator is bank-aligned
        nc.psum_tensor([1, N_PS, 512], mybir.dt.float32) as pst,
        nc.semaphore("s_init") as s_init,
        # input-completion sems split by block parity: a consumer threshold
        # then only ever counts DMAs of blocks <= v of that parity (blocks
        # v+2 of the same parity are gated on adds of block v), so a
        # straggler from an adjacent block can never satisfy the wait.
        nc.semaphore("s_in0") as s_in0,
        nc.semaphore("s_in1") as s_in1,
        nc.semaphore("s_it0") as s_it0,
        nc.semaphore("s_it1") as s_it1,
        nc.semaphore("s_add") as s_add,
        nc.semaphore("s_pe") as s_pe,
        nc.semaphore("s_cp") as s_cp,
        nc.semaphore("s_out") as s_out,
        nc.Block() as block,
    ):
        s_in = [s_in0, s_in1]
        s_it = [s_it0, s_it1]

        def tslot(v, k):
            return tbuf[:, ((v % 2) * N_K + k) * F_DMA : ((v % 2) * N_K + k + 1) * F_DMA]

        def kp_of(k):
            return min(P, N_CLUSTERS - k * P)

        # clusters 0:896 of a block, as one DMA: DRAM rows (k*128+p) map to
        # SBUF (p, k*2500+w), which is contiguous free-dim on the SBUF side
        adj_kp = adj_s[: (N_K - 1) * P, :].rearrange("(k p) w -> p k w", p=P)

        @block.sync
        def _(sync):
            for v in range(NV):
                vb = v % N_VB
                if v >= 2:
                    # slots (v%2, *) free once DVE summed all of block v-2
                    sync.wait_ge(s_add, v - 1)
                # k = 0..6 fused (896 rows), then the 104-row tail chunk
                sync.dma_start(
                    out=tbuf[:, (v % 2) * N_K * F_DMA : ((v % 2) * N_K + N_K - 1) * F_DMA],
                    in_=adj_kp[:, 0 : N_K - 1, vb * F_DMA : (vb + 1) * F_DMA],
                ).then_inc(s_in[v % 2], 16)
                kp = kp_of(N_K - 1)
                sync.dma_start(
                    out=tslot(v, N_K - 1)[:kp, :],
                    in_=adj_s[(N_K - 1) * P : (N_K - 1) * P + kp, vb * F_DMA : (vb + 1) * F_DMA],
                ).then_inc(s_it[v % 2], 16)

        @block.gpsimd
        def _(gpsimd):
            for v in range(NV):
                vb = v % N_VB
                gpsimd.wait_ge(s_cp, N_I * (v + 1))
                gpsimd.dma_start(
                    out=cs[vb * F_DMA : (vb + 1) * F_DMA],
                    in_=obuf[:1, (v % 2) * F_DMA : (v % 2 + 1) * F_DMA],
                ).then_inc(s_out, 16)
            gpsimd.wait_ge(s_out, 16 * NV)

        @block.tensor
        def _(tensor):
            tensor.wait_ge(s_init, 1)  # ones ready
            for v in range(NV):
                # acc for block v complete
                tensor.wait_ge(s_add, v + 1)
                for i in range(N_I):
                    g = v * N_I + i
                    if g >= N_PS:
                        # PSUM bank g%N_PS free once copy of group g-N_PS done
                        tensor.wait_ge(s_cp, g - N_PS + 1)
                    nc.tensor.matmul(
                        pst[:1, g % N_PS, :F_MM],
                        ones[:, :1],
                        acc[:, i * F_MM : (i + 1) * F_MM],
                        start=True,
                        stop=True,
                    ).then_inc(s_pe, 1)

        @block.vector
        def _(vector):
            vector.memset(ones[:, :], 1.0).then_inc(s_init, 1)
            for v in range(NV):
                # acc free once PE consumed block v-1's acc
                if v >= 1:
                    vector.wait_ge(s_pe, N_I * v)
                # fused chunk (tiles 0..6) arrived; tail has its own sem so
                # its completion can never satisfy this count
                vector.wait_ge(s_in[v % 2], 16 * (v // 2 + 1))
                # acc = sum of the 8 cluster chunks (tail chunk is 104 rows)
                add = nc.vector.tensor_add(acc[:, :], tslot(v, 0)[:, :], tslot(v, 1)[:, :])
                for k in range(2, N_K - 1):
                    add = nc.vector.tensor_add(
                        acc[:, :], acc[:, :], tslot(v, k)[:, :]
                    )
                vector.wait_ge(s_it[v % 2], 16 * (v // 2 + 1))
                kp = kp_of(N_K - 1)
                add = nc.vector.tensor_add(
                    acc[:kp, :], acc[:kp, :], tslot(v, N_K - 1)[:kp, :]
                )
                add.then_inc(s_add, 1)
                for i in range(N_I):
                    g = v * N_I + i
                    if i == 0 and v >= 2:
                        # obuf half (v%2) free once output DMA of v-2 done
                        vector.wait_ge(s_out, 16 * (v - 1))
                    vector.wait_ge(s_pe, g + 1)
                    nc.vector.tensor_copy(
                        obuf[:1, (v % 2) * F_DMA + i * F_MM : (v % 2) * F_DMA + (i + 1) * F_MM],
                        pst[:1, g % N_PS, :F_MM],
                    ).then_inc(s_cp, 1)

    return nc


def kernel(x, adj, att, key_w):
    adj = np.ascontiguousarray(np.asarray(adj), dtype=np.float32)
    assert adj.shape == (N_CLUSTERS, N_VERTICES)

    nc = _build_nc()
    in_maps = [
        {"adj_s": np.ascontiguousarray(adj[:, i * V_SHARD : (i + 1) * V_SHARD])}
        for i in range(N_CORES)
    ]
    res = run_bass_kernel_spmd(nc, in_maps, core_ids=list(range(N_CORES)))
    colsum = np.concatenate([r["cs"] for r in res.results]).astype(np.float32)

    with np.errstate(divide="ignore", invalid="ignore"):
        att_vertices = (colsum / colsum).reshape(N_VERTICES, 1).astype(np.float32)
    att_clusters = np.ones((N_CLUSTERS, 1), dtype=np.float32)
    return att_vertices, att_clusters


# revision 25
# speedup vs baseline: 805.0154x; 1.1213x over previous
"""ClusterAttention2 Trainium2 kernel.

Mathematical simplification: the reference computes
    logits       : [n_clusters, 1]
    att_clusters = softmax(logits, axis=1)   # axis of size 1 -> exactly ones
    att_vertices = adj.T @ att_clusters      # == per-vertex column sum of adj
    att_vertices = att_vertices / max(att_vertices, axis=1)  # [N,1] -> x/x
so for any finite logits the output is exactly
    att_clusters = ones([n_clusters, 1])
    att_vertices = colsum / colsum           # 1.0, or NaN where colsum == 0
The only data-dependent work is the column sum of adj (400 MB -> memory
bound).  Each of the 8 cores reads its [1000, 12500] vertex-shard of adj
(50 MB, the per-core HBM roofline at ~358 GB/s is ~140 us) and reduces the
cluster dimension in two steps: DVE sums the 8 row-chunks of 128 clusters
lane-wise into acc[128, 2500], then one tensor-engine matmul per 500
columns (ones[128,1].T @ acc) folds the remaining 128 partitions.  The
final x/x division (IEEE 0/0 -> NaN) runs on the host so NaN positions
match the reference bit-for-bit.

Written in raw Bass (explicit semaphores): the TPB ISA allows a single
semaphore wait per instruction (this walrus build hard-errors on more), so
every wait is a standalone wait_ge on the consuming engine, never attached
to a data instruction.

Pipeline (per core), vertex blocks v of width 2500 (5 per pass):
  SP   : 3 input DMAs per block into sbuf half (v%2) — fused transfers
         for cluster rows 0:384 and 384:896 (DRAM rows k*128+p map to
         SBUF (p, k*2500+w), contiguous on the SBUF side) and one for the
         104-row tail; gated on s_add so a half is only overwritten after
         DVE consumed it.
  DVE  : acc = sum of the block's 8 chunks (7 tensor_adds; each DMA piece
         has its own parity-split completion sem so a count can never be
         satisfied by a different DMA finishing first), then copy each
         finished PSUM group into obuf[v%2].
  PE   : 5 single matmuls per block (K=128) into a rotating PSUM bank.
  POOL : output DMA obuf[v%2] -> cs per block, gated on s_cp.

`repeat` > 1 replays the whole pipeline (same data, same output) for
slope-based hardware timing through the high-overhead dispatch path.
"""

import numpy as np

import concourse.bass as bass
import concourse.mybir as mybir
from concourse.bass_utils import run_bass_kernel_spmd

N_CLUSTERS = 1000
N_VERTICES = 100000
N_CORES = 8
V_SHARD = N_VERTICES // N_CORES  # 12500 vertices per core
P = 128                          # cluster chunk (partition dim)
N_K = (N_CLUSTERS + P - 1) // P  # 8 chunks: 7x128 + 104
F_DMA = 2500                     # vertices per DMA tile (128x2500 f32 = 1.25 MB)
F_MM = 500                       # vertices per matmul (PSUM bank = 512 f32)
N_VB = V_SHARD // F_DMA          # 5 vertex blocks
N_I = F_DMA // F_MM              # 5 accumulation groups per block
N_PS = 4                         # rotating PSUM banks


def _build_nc(repeat: int = 1) -> bass.Bass:
    nc = bass.Bass()
    adj_s = nc.dram_tensor(
        "adj_s", [N_CLUSTERS, V_SHARD], mybir.dt.float32, kind="ExternalInput"
    )
    cs = nc.dram_tensor("cs", [V_SHARD], mybir.dt.float32, kind="ExternalOutput")
    NV = N_VB * repeat  # global vertex-block count

    with (
        nc.sbuf_tensor([P, 2 * N_K * F_DMA], mybir.dt.float32) as tbuf,
        nc.sbuf_tensor([P, F_DMA], mybir.dt.float32) as acc,
        nc.sbuf_tensor([P, 1], mybir.dt.float32) as ones,
        nc.sbuf_tensor([1, 2 * F_DMA], mybir.dt.float32) as obuf,
        # 512-f32 stride so each rotating accumulator is bank-aligned
        nc.psum_tensor([1, N_PS, 512], mybir.dt.float32) as pst,
        nc.semaphore("s_init") as s_init,
        # input-completion sems split by block parity: a consumer threshold
        # then only ever counts DMAs of blocks <= v of that parity (blocks
        # v+2 of the same parity are gated on adds of block v), so a
        # straggler from an adjacent block can never satisfy the wait.
        nc.semaphore("s_inA0") as s_inA0,
        nc.semaphore("s_inA1") as s_inA1,
        nc.semaphore("s_inB0") as s_inB0,
        nc.semaphore("s_inB1") as s_inB1,
        nc.semaphore("s_it0") as s_it0,
        nc.semaphore("s_it1") as s_it1,
        nc.semaphore("s_add") as s_add,
        nc.semaphore("s_pe") as s_pe,
        nc.semaphore("s_cp") as s_cp,
        nc.semaphore("s_out") as s_out,
        nc.Block() as block,
    ):
        s_inA = [s_inA0, s_inA1]
        s_inB = [s_inB0, s_inB1]
        s_it = [s_it0, s_it1]
        N_A = 3  # chunks k=0..2 in piece A, k=3..6 in piece B, k=7 tail

        def tslot(v, k):
            return tbuf[:, ((v % 2) * N_K + k) * F_DMA : ((v % 2) * N_K + k + 1) * F_DMA]

        def kp_of(k):
            return min(P, N_CLUSTERS - k * P)

        # clusters 0:896 of a block, as one DMA: DRAM rows (k*128+p) map to
        # SBUF (p, k*2500+w), which is contiguous free-dim on the SBUF side
        adj_kp = adj_s[: (N_K - 1) * P, :].rearrange("(k p) w -> p k w", p=P)

        @block.sync
        def _(sync):
            for v in range(NV):
                vb = v % N_VB
                if v >= 2:
                    # slots (v%2, *) free once DVE summed all of block v-2
                    sync.wait_ge(s_add, v - 1)
                # fused rows in two pieces (k=0..2, k=3..6) so the DVE add
                # chain can start before the whole block lands, then the
                # 104-row tail chunk
                sync.dma_start(
                    out=tbuf[:, (v % 2) * N_K * F_DMA : ((v % 2) * N_K + N_A) * F_DMA],
                    in_=adj_kp[:, 0:N_A, vb * F_DMA : (vb + 1) * F_DMA],
                ).then_inc(s_inA[v % 2], 16)
                sync.dma_start(
                    out=tbuf[:, ((v % 2) * N_K + N_A) * F_DMA : ((v % 2) * N_K + N_K - 1) * F_DMA],
                    in_=adj_kp[:, N_A : N_K - 1, vb * F_DMA : (vb + 1) * F_DMA],
                ).then_inc(s_inB[v % 2], 16)
                kp = kp_of(N_K - 1)
                sync.dma_start(
                    out=tslot(v, N_K - 1)[:kp, :],
                    in_=adj_s[(N_K - 1) * P : (N_K - 1) * P + kp, vb * F_DMA : (vb + 1) * F_DMA],
                ).then_inc(s_it[v % 2], 16)

        @block.gpsimd
        def _(gpsimd):
            for v in range(NV):
                vb = v % N_VB
                gpsimd.wait_ge(s_cp, N_I * (v + 1))
                gpsimd.dma_start(
                    out=cs[vb * F_DMA : (vb + 1) * F_DMA],
                    in_=obuf[:1, (v % 2) * F_DMA : (v % 2 + 1) * F_DMA],
                ).then_inc(s_out, 16)
            gpsimd.wait_ge(s_out, 16 * NV)

        @block.tensor
        def _(tensor):
            tensor.wait_ge(s_init, 1)  # ones ready
            for v in range(NV):
                # acc for block v complete
                tensor.wait_ge(s_add, v + 1)
                for i in range(N_I):
                    g = v * N_I + i
                    if g >= N_PS:
                        # PSUM bank g%N_PS free once copy of group g-N_PS done
                        tensor.wait_ge(s_cp, g - N_PS + 1)
                    nc.tensor.matmul(
                        pst[:1, g % N_PS, :F_MM],
                        ones[:, :1],
                        acc[:, i * F_MM : (i + 1) * F_MM],
                        start=True,
                        stop=True,
                    ).then_inc(s_pe, 1)

        @block.vector
        def _(vector):
            vector.memset(ones[:, :], 1.0).then_inc(s_init, 1)
            for v in range(NV):
                # acc free once PE consumed block v-1's acc
                if v >= 1:
                    vector.wait_ge(s_pe, N_I * v)
                # each piece has its own parity-split sem so a count can
                # never be satisfied by a different DMA finishing first
                vector.wait_ge(s_inA[v % 2], 16 * (v // 2 + 1))
                # acc = sum of the 8 cluster chunks (tail chunk is 104 rows)
                add = nc.vector.tensor_add(acc[:, :], tslot(v, 0)[:, :], tslot(v, 1)[:, :])
                for k in range(2, N_A):
                    add = nc.vector.tensor_add(
                        acc[:, :], acc[:, :], tslot(v, k)[:, :]
                    )
                vector.wait_ge(s_inB[v % 2], 16 * (v // 2 + 1))
                for k in range(N_A, N_K - 1):
                    add = nc.vector.tensor_add(
                        acc[:, :], acc[:, :], tslot(v, k)[:, :]
                    )
                vector.wait_ge(s_it[v % 2], 16 * (v // 2 + 1))
                kp = kp_of(N_K - 1)
                add = nc.vector.tensor_add(
                    acc[:kp, :], acc[:kp, :], tslot(v, N_K - 1)[:kp, :]
                )
                add.then_inc(s_add, 1)
                for i in range(N_I):
                    g = v * N_I + i
                    if i == 0 and v >= 2:
                        # obuf half (v%2) free once output DMA of v-2 done
                        vector.wait_ge(s_out, 16 * (v - 1))
                    vector.wait_ge(s_pe, g + 1)
                    nc.vector.tensor_copy(
                        obuf[:1, (v % 2) * F_DMA + i * F_MM : (v % 2) * F_DMA + (i + 1) * F_MM],
                        pst[:1, g % N_PS, :F_MM],
                    ).then_inc(s_cp, 1)

    return nc


def kernel(x, adj, att, key_w):
    adj = np.ascontiguousarray(np.asarray(adj), dtype=np.float32)
    assert adj.shape == (N_CLUSTERS, N_VERTICES)

    nc = _build_nc()
    in_maps = [
        {"adj_s": np.ascontiguousarray(adj[:, i * V_SHARD : (i + 1) * V_SHARD])}
        for i in range(N_CORES)
    ]
    res = run_bass_kernel_spmd(nc, in_maps, core_ids=list(range(N_CORES)))
    colsum = np.concatenate([r["cs"] for r in res.results]).astype(np.float32)

    with np.errstate(divide="ignore", invalid="ignore"):
        att_vertices = (colsum / colsum).reshape(N_VERTICES, 1).astype(np.float32)
    att_clusters = np.ones((N_CLUSTERS, 1), dtype=np.float32)
    return att_vertices, att_clusters
